# revision 22
# baseline (speedup 1.0000x reference)
"""GQA multi-head attention (B=2, S=2048, E=2048, 32 q-heads, 8 kv-heads) on 8 TRN2 cores.

Sharding: tensor-parallel over kv-heads (core c owns kv-head c and query heads
4c..4c+3 for both batches). After attention, a per-batch AllToAll re-shards the
context from head-sharding to token-sharding; each core then runs the output
projection for its 256-token slice of each batch against the full Wo. The host
gather is pure concatenation.

All heavy matmuls run as float32r (full-rate PE, ~1e-4 rel err). Causality is
structural: above-diagonal score tiles are skipped, diagonal 128x128 blocks are
masked with a single precomputed triangular 0/1 mask. Softmax runs without
max-subtraction (causal scores are within [-12, 11] for this problem family);
row sums come from a ones-column appended to V inside the PV matmul, and the
1/sum normalization is broadcast across partitions with a K=1 matmul.
"""

import numpy as np

B = 2
S = 2048
E = 2048
KV = 8
G = 4
D = 64
H = 32
N_CORES = 8
KC = E // 128  # 16 contraction chunks
NT = S // 512  # 4 q-tiles of 512
NQ = 4         # token quarters for projections (512 each)
TOK = 256      # per-core token slice per batch (out proj)
NTB = S // 128  # 16 token blocks of 128

_CACHE = {}


def _build_nc():
    import concourse.mybir as mybir
    import concourse.tile as tile
    from concourse import bacc
    f32 = mybir.dt.float32
    f32r = mybir.dt.float32r
    bf16 = mybir.dt.bfloat16
    Exp = mybir.ActivationFunctionType.Exp

    nc = bacc.Bacc(target_bir_lowering=False, num_devices=N_CORES)

    xT = nc.dram_tensor("xT", [B, KC, 128, S], f32r, kind="ExternalInput")
    wqkvT = nc.dram_tensor("wqkvT", [KC, 128, 384], f32r, kind="ExternalInput")
    woT = nc.dram_tensor("woT", [KC, 128, E], f32r, kind="ExternalInput")
    misc = nc.dram_tensor("misc", [128, 320], f32r, kind="ExternalInput")
    out = nc.dram_tensor("out", [B * TOK, E], f32, kind="ExternalOutput")

    with tile.TileContext(nc) as tc:
        with tc.tile_pool(name="const", bufs=1) as const, \
             tc.tile_pool(name="mm_ps", bufs=1, space="PSUM") as mm_ps, \
             tc.tile_pool(name="dram", bufs=1, space="DRAM") as dram:

            # ---- constants ----
            misc_sb = const.tile([128, 320], f32r, name="misc_sb")
            nc.sync.dma_start(misc_sb[:], misc[:])
            tri_sb = misc_sb[:, 0:128]
            ident = misc_sb[:, 128:256]
            ones_col = misc_sb[:, 256:257]
            ones65 = misc_sb[:, 256:320]  # any all-ones block

            # ---- collective buffers (per batch) ----
            cc_in = [[dram.tile([N_CORES, 128, TOK], f32r, name=f"cc_in{b}{p}")
                      for p in range(2)] for b in range(B)]
            cc_out = [[dram.tile([N_CORES, 128, TOK], f32r, name=f"cc_out{b}{p}")
                       for p in range(2)] for b in range(B)]

            ctx_heads = {}  # (b, h) -> [64, S] sbuf tile

            def phase1(b, sbA, xpool):
                """Projections for batch b -> q01/q23/kv sbuf tiles [128, S]."""
                q01 = sbA.tile([128, S], f32r, name=f"q01_{b}", tag="qkvT", bufs=3)
                q23 = sbA.tile([128, S], f32r, name=f"q23_{b}", tag="qkvT", bufs=3)
                kv = sbA.tile([128, S], f32r, name=f"kv_{b}", tag="qkvT", bufs=3)
                dst = [q01, q23, kv]
                for tq in range(NQ):  # token quarters of 512
                    xq = xpool.tile([128, KC * 512], f32r, name=f"x{b}{tq}",
                                    tag="x", bufs=2)
                    for kc in range(KC):
                        nc.sync.dma_start(
                            xq[:, kc * 512:(kc + 1) * 512],
                            xT[b, kc, :, tq * 512:(tq + 1) * 512])
                    for mc in range(3):
                        ps = mm_ps.tile([128, 512], f32, name=f"p{b}{tq}{mc}", tag="mm")
                        for kc in range(KC):
                            nc.tensor.matmul(
                                ps[:],
                                w_sb[:, kc * 384 + mc * 128: kc * 384 + (mc + 1) * 128],
                                xq[:, kc * 512:(kc + 1) * 512],
                                start=(kc == 0), stop=(kc == KC - 1),
                            )
                        nc.vector.tensor_copy(
                            dst[mc][:, tq * 512:(tq + 1) * 512], ps[:])
                return q01, q23, kv

            def phase15(b, kv, sbA, rb_psp):
                """kdup (k on both partition halves) + token-major v_aug tiles."""
                kdup = sbA.tile([128, S], f32r, name=f"kdup{b}", tag="kdup", bufs=1)
                # kv rows 64:128 hold k (layout [v; k])
                nc.sync.dma_start(kdup[0:64, :], kv[64:128, :])
                nc.sync.dma_start(kdup[64:128, :], kv[64:128, :])
                va = sbA.tile([128, NTB * 65], f32r, name=f"va{b}", tag="vaug", bufs=2)
                for t in range(NTB):
                    tp = rb_psp.tile([128, 64], f32r, name=f"vt{b}{t}", tag="rbvt")
                    nc.tensor.transpose(tp[:], kv[0:64, t * 128:(t + 1) * 128],
                                        misc_sb[0:64, 128:192])
                    nc.vector.tensor_copy(va[:, t * 65:t * 65 + 64], tp[:])
                    nc.vector.tensor_copy(va[:, t * 65 + 64:t * 65 + 65], ones_col)
                return kdup, va

            def attention(b, q01, q23, kdup, va, sbA, sc_ps, cx_ps, rb_psp, stage):
                qpair = [q01, q01, q23, q23]
                for h in range(G):
                    base = 64 * (h % 2)
                    ch = sbA.tile([64, S], f32r, name=f"ctx{b}{h}", tag="ctxh", bufs=4)
                    ctx_heads[(b, h)] = ch
                    for qt in range(NT):
                        cps = cx_ps.tile([65, 512], f32, name=f"c{b}{h}{qt}", tag="cx")
                        nkb = 4 * qt + 4
                        for kb in range(nkb):
                            r = kb - 4 * qt
                            off = 128 * r if r >= 0 else 0
                            w = 512 - off
                            sps = sc_ps.tile([128, 512], f32, name=f"s{b}{h}{qt}{kb}",
                                             tag="sc")
                            nc.tensor.matmul(
                                sps[:, 0:w],
                                kdup[base:base + 64, kb * 128:(kb + 1) * 128],
                                qpair[h][base:base + 64,
                                         qt * 512 + off: qt * 512 + off + w],
                                start=True, stop=True,
                            )
                            if r >= 0:
                                nc.vector.tensor_add(sps[:, 0:128], sps[:, 0:128],
                                                     tri_sb)
                            ex = sbA.tile([128, 512], f32r, name=f"e{b}{h}{qt}{kb}",
                                          tag="exp", bufs=4)
                            nc.scalar.activation(ex[:, 0:w], sps[:, 0:w], Exp,
                                                 scale=0.125)
                            nc.tensor.matmul(
                                cps[:, off:off + w],
                                va[:, kb * 65:(kb + 1) * 65],
                                ex[:, 0:w],
                                start=(kb == 0), stop=(kb == nkb - 1),
                            )
                        # softmax normalization: recip of sums (row 64), bcast, mult
                        rc = sbA.tile([65, 512], f32r, name=f"rc{b}{h}{qt}", tag="rc",
                                      bufs=2)
                        with nc.allow_low_precision(reason="f32r softmax recip"):
                            nc.vector.reciprocal(rc[64:65, :], cps[64:65, :])
                        rbp = rb_psp.tile([64, 512], f32, name=f"rb{b}{h}{qt}",
                                          tag="rbvt")
                        nc.tensor.matmul(rbp[:], ones65[64:65, :], rc[64:65, :],
                                         start=True, stop=True)
                        rbs = sbA.tile([64, 512], f32, name=f"rbs{b}{h}{qt}", tag="rbs",
                                       bufs=2)
                        nc.vector.tensor_copy(rbs[:], rbp[:])
                        nc.vector.tensor_mul(ch[:, qt * 512:(qt + 1) * 512],
                                             cps[0:64, :], rbs[:])
                    if h == 1:
                        stage(b, 0)
                    elif h == 3:
                        stage(b, 1)

            def stage_a2a(b, p):
                # stage + exchange heads 2p, 2p+1 (f rows p*128:(p+1)*128 of each shard)
                for j in range(N_CORES):
                    for lane in range(2):
                        h = 2 * p + lane
                        nc.sync.dma_start(
                            cc_in[b][p][j, lane * 64:(lane + 1) * 64, :],
                            ctx_heads[(b, h)][:, j * TOK:(j + 1) * TOK],
                        )
                nc.gpsimd.collective_compute(
                    "AllToAll",
                    mybir.AluOpType.bypass,
                    replica_groups=[list(range(N_CORES))],
                    ins=[cc_in[b][p][:]],
                    outs=[cc_out[b][p][:]],
                )

            with tc.tile_pool(name="sbA", bufs=1) as sbA, \
                 tc.tile_pool(name="wp", bufs=1) as wpool, \
                 tc.tile_pool(name="xp", bufs=1) as xpool, \
                 tc.tile_pool(name="sc_ps", bufs=4, space="PSUM") as sc_ps, \
                 tc.tile_pool(name="cx_ps", bufs=2, space="PSUM") as cx_ps, \
                 tc.tile_pool(name="rb_ps", bufs=1, space="PSUM") as rb_psp:
                w_sb = wpool.tile([128, KC * 384], f32r, name="w_sb")
                for kc in range(KC):
                    nc.sync.dma_start(w_sb[:, kc * 384:(kc + 1) * 384], wqkvT[kc])
                q01, q23, kv0 = phase1(0, sbA, xpool)
                kdup0, va0 = phase15(0, kv0, sbA, rb_psp)
                attention(0, q01, q23, kdup0, va0, sbA, sc_ps, cx_ps, rb_psp,
                          stage_a2a)
                q01b, q23b, kv1 = phase1(1, sbA, xpool)
                kdup1, va1 = phase15(1, kv1, sbA, rb_psp)
                attention(1, q01b, q23b, kdup1, va1, sbA, sc_ps, cx_ps, rb_psp,
                          stage_a2a)

            # ---- out projection for local token slices ----
            # Streamed Wo; batch 0 fully emitted first so its matmuls and Wo
            # prefetch cover the A2A of batch 1.
            with tc.tile_pool(name="sbB", bufs=1) as sbB, \
                 tc.tile_pool(name="op_ps", bufs=4, space="PSUM") as op_ps:
                ctx_sb = {}
                for b in range(B):
                    t = sbB.tile([128, KC * TOK], f32r, name=f"cc{b}", tag="ccr", bufs=2)
                    for fc in range(KC):
                        nc.sync.dma_start(
                            t[:, fc * TOK:(fc + 1) * TOK],
                            cc_out[b][fc % 2][fc // 2, :, :])
                    ctx_sb[b] = t
                for bb in range(B):
                    for et in range(4):
                        pss = {}
                        for mt in range(TOK // 128):
                            pss[mt] = op_ps.tile([128, 512], f32, name=f"o{bb}{et}{mt}",
                                                 tag="op")
                        for fc in range(KC):
                            wo = sbB.tile([128, 512], f32r, name=f"wo{bb}{et}{fc}",
                                          tag="wo", bufs=4)
                            nc.sync.dma_start(wo[:], woT[fc, :, et * 512:(et + 1) * 512])
                            for mt in range(TOK // 128):
                                nc.tensor.matmul(
                                    pss[mt][:],
                                    ctx_sb[bb][:, fc * TOK + mt * 128: fc * TOK + (mt + 1) * 128],
                                    wo[:],
                                    start=(fc == 0), stop=(fc == KC - 1),
                                )
                        for mt in range(TOK // 128):
                            os_ = sbB.tile([128, 512], f32, name=f"os{bb}{et}{mt}",
                                           tag="os", bufs=2)
                            nc.vector.tensor_copy(os_[:], pss[mt][:])
                            nc.sync.dma_start(
                                out[bb * TOK + mt * 128: bb * TOK + (mt + 1) * 128,
                                    et * 512:(et + 1) * 512],
                                os_[:],
                            )

    nc.compile()
    return nc


def _prep_inputs(x, Wq, Wk, Wv, Wo):
    """Host-side sharding/transposes. Returns per-core in_maps."""
    xT = np.ascontiguousarray(
        x.transpose(0, 2, 1)).reshape(B, KC, 128, S).astype(np.float32)
    woT = np.ascontiguousarray(Wo.T).reshape(KC, 128, E).astype(np.float32)
    # additive causal mask for transposed scores: 0 where q>=kv (j>=i), -1e30 above
    tri = np.where(np.triu(np.ones((128, 128), dtype=np.float32)) > 0, 0.0,
                   -1e30).astype(np.float32)
    ident = np.eye(128, dtype=np.float32)
    ones = np.ones((128, 64), dtype=np.float32)
    misc = np.ascontiguousarray(np.concatenate([tri, ident, ones], axis=1))
    in_maps = []
    for c in range(N_CORES):
        wc = np.concatenate([
            Wq[256 * c:256 * (c + 1)],          # q heads 4c..4c+3 -> cols 0..255
            Wv[64 * c:64 * (c + 1)],            # v                -> cols 256..319
            Wk[64 * c:64 * (c + 1)],            # k                -> cols 320..383
        ], axis=0)                              # [384, E]
        wqkvT = np.ascontiguousarray(wc.T).reshape(KC, 128, 384).astype(np.float32)
        in_maps.append({"xT": xT, "wqkvT": wqkvT, "woT": woT, "misc": misc})
    return in_maps


def _ensure_ntff_hook():
    """Install antenv.axon_hooks shim so trace=True can capture NTFF profiles."""
    import sys
    import types
    try:
        from antenv.axon_hooks import get_axon_ntff_profile_hook  # noqa: F401
        return
    except ImportError:
        pass
    mod = types.ModuleType("antenv.axon_hooks")
    _h = [None]
    mod.set_axon_ntff_profile_hook = lambda h: _h.__setitem__(0, h)
    mod.get_axon_ntff_profile_hook = lambda: _h[0]
    sys.modules["antenv.axon_hooks"] = mod
    try:
        from trn_agent_boot.trn_boot import _ntff_profile_via_ctypes
        hook = _ntff_profile_via_ctypes("/opt/axon/libaxon_pjrt.so")
        if hook is not None:
            mod.set_axon_ntff_profile_hook(hook)
    except Exception:
        pass


def kernel(x, mask, Wq, Wk, Wv, Wo, trace=False):
    from concourse.bass_utils import run_bass_kernel_spmd
    if trace:
        _ensure_ntff_hook()

    x = np.asarray(x, dtype=np.float32)
    Wq = np.asarray(Wq, dtype=np.float32)
    Wk = np.asarray(Wk, dtype=np.float32)
    Wv = np.asarray(Wv, dtype=np.float32)
    Wo = np.asarray(Wo, dtype=np.float32)

    if "nc" not in _CACHE:
        _CACHE["nc"] = _build_nc()
    nc = _CACHE["nc"]

    in_maps = _prep_inputs(x, Wq, Wk, Wv, Wo)
    res = run_bass_kernel_spmd(nc, in_maps, core_ids=list(range(N_CORES)),
                               trace=trace)
    _CACHE["last_result"] = res

    full = np.empty((B, S, E), dtype=np.float32)
    for c in range(N_CORES):
        o = res.results[c]["out"]  # [B*TOK, E]
        for b in range(B):
            full[b, TOK * c:TOK * (c + 1), :] = o[b * TOK:(b + 1) * TOK]
    return full


# revision 25
# speedup vs baseline: 1.1345x; 1.1345x over previous
"""GQA multi-head attention (B=2, S=2048, E=2048, 32 q-heads, 8 kv-heads) on 8 TRN2 cores.

Sharding: tensor-parallel over kv-heads (core c owns kv-head c and query heads
4c..4c+3 for both batches). After attention, a per-batch AllToAll re-shards the
context from head-sharding to token-sharding; each core then runs the output
projection for its 256-token slice of each batch against the full Wo. The host
gather is pure concatenation.

All heavy matmuls run as float32r (full-rate PE, ~1e-4 rel err). Causality is
structural: above-diagonal score tiles are skipped, diagonal 128x128 blocks are
masked with a single precomputed triangular 0/1 mask. Softmax runs without
max-subtraction (causal scores are within [-12, 11] for this problem family);
row sums come from a ones-column appended to V inside the PV matmul, and the
1/sum normalization is broadcast across partitions with a K=1 matmul.
"""

import numpy as np

B = 2
S = 2048
E = 2048
KV = 8
G = 4
D = 64
H = 32
N_CORES = 8
KC = E // 128  # 16 contraction chunks
NT = S // 512  # 4 q-tiles of 512
NQ = 4         # token quarters for projections (512 each)
TOK = 256      # per-core token slice per batch (out proj)
NTB = S // 128  # 16 token blocks of 128

_CACHE = {}


def _build_nc():
    import concourse.mybir as mybir
    import concourse.tile as tile
    from concourse import bacc
    f32 = mybir.dt.float32
    f32r = mybir.dt.float32r
    bf16 = mybir.dt.bfloat16
    Exp = mybir.ActivationFunctionType.Exp

    nc = bacc.Bacc(target_bir_lowering=False, num_devices=N_CORES)

    xT = nc.dram_tensor("xT", [B, KC, 128, S], f32r, kind="ExternalInput")
    wqkvT = nc.dram_tensor("wqkvT", [KC, 128, 384], f32r, kind="ExternalInput")
    woT = nc.dram_tensor("woT", [KC, 128, E], f32r, kind="ExternalInput")
    misc = nc.dram_tensor("misc", [128, 320], f32r, kind="ExternalInput")
    out = nc.dram_tensor("out", [B * TOK, E], f32, kind="ExternalOutput")

    with tile.TileContext(nc) as tc:
        with tc.tile_pool(name="const", bufs=1) as const, \
             tc.tile_pool(name="mm_ps", bufs=1, space="PSUM") as mm_ps, \
             tc.tile_pool(name="dram", bufs=1, space="DRAM") as dram:

            # ---- constants ----
            misc_sb = const.tile([128, 320], f32r, name="misc_sb")
            nc.sync.dma_start(misc_sb[:], misc[:])
            tri_sb = misc_sb[:, 0:128]
            ident = misc_sb[:, 128:256]
            ones_col = misc_sb[:, 256:257]
            ones65 = misc_sb[:, 256:320]  # any all-ones block

            # ---- collective buffers (per batch) ----
            cc_in = [[dram.tile([N_CORES, 128, TOK], f32r, name=f"cc_in{b}{p}")
                      for p in range(2)] for b in range(B)]
            cc_out = [[dram.tile([N_CORES, 128, TOK], f32r, name=f"cc_out{b}{p}")
                       for p in range(2)] for b in range(B)]

            ctx_heads = {}  # (b, h) -> [64, S] sbuf tile

            def phase1(b, sbA, xpool):
                """Projections for batch b -> q01/q23/kv sbuf tiles [128, S]."""
                q01 = sbA.tile([128, S], f32r, name=f"q01_{b}", tag="qkvT", bufs=3)
                q23 = sbA.tile([128, S], f32r, name=f"q23_{b}", tag="qkvT", bufs=3)
                kv = sbA.tile([128, S], f32r, name=f"kv_{b}", tag="qkvT", bufs=3)
                dst = [q01, q23, kv]
                for tq in range(NQ):  # token quarters of 512
                    xq = xpool.tile([128, KC * 512], f32r, name=f"x{b}{tq}",
                                    tag="x", bufs=2)
                    for kc in range(KC):
                        nc.sync.dma_start(
                            xq[:, kc * 512:(kc + 1) * 512],
                            xT[b, kc, :, tq * 512:(tq + 1) * 512])
                    for mc in range(3):
                        ps = mm_ps.tile([128, 512], f32, name=f"p{b}{tq}{mc}", tag="mm")
                        for kc in range(KC):
                            nc.tensor.matmul(
                                ps[:],
                                w_sb[:, kc * 384 + mc * 128: kc * 384 + (mc + 1) * 128],
                                xq[:, kc * 512:(kc + 1) * 512],
                                start=(kc == 0), stop=(kc == KC - 1),
                            )
                        nc.vector.tensor_copy(
                            dst[mc][:, tq * 512:(tq + 1) * 512], ps[:])
                return q01, q23, kv

            def phase15(b, kv, sbA, rb_psp):
                """kdup (k on both partition halves) + token-major v_aug tiles."""
                kdup = sbA.tile([128, S], f32r, name=f"kdup{b}", tag="kdup", bufs=1)
                # kv rows 64:128 hold k (layout [v; k])
                nc.sync.dma_start(kdup[0:64, :], kv[64:128, :])
                nc.sync.dma_start(kdup[64:128, :], kv[64:128, :])
                va = sbA.tile([128, NTB * 65], f32r, name=f"va{b}", tag="vaug", bufs=1)
                for t in range(NTB):
                    tp = rb_psp.tile([128, 64], f32r, name=f"vt{b}{t}", tag="rbvt")
                    nc.tensor.transpose(tp[:], kv[0:64, t * 128:(t + 1) * 128],
                                        misc_sb[0:64, 128:192])
                    nc.vector.tensor_copy(va[:, t * 65:t * 65 + 64], tp[:])
                    nc.vector.tensor_copy(va[:, t * 65 + 64:t * 65 + 65], ones_col)
                return kdup, va

            def attention(b, q01, q23, kdup, va, sbA, sc_ps, cx_ps, rb_psp, stage):
                qpair = [q01, q01, q23, q23]
                for h in range(G):
                    base = 64 * (h % 2)
                    ch = sbA.tile([64, S], f32r, name=f"ctx{b}{h}", tag="ctxh", bufs=3)
                    ctx_heads[(b, h)] = ch
                    for qt in range(NT):
                        cps = cx_ps.tile([65, 512], f32, name=f"c{b}{h}{qt}", tag="cx")
                        nkb = 4 * qt + 4
                        for kb in range(nkb):
                            r = kb - 4 * qt
                            off = 128 * r if r >= 0 else 0
                            w = 512 - off
                            sps = sc_ps.tile([128, 512], f32, name=f"s{b}{h}{qt}{kb}",
                                             tag="sc")
                            nc.tensor.matmul(
                                sps[:, 0:w],
                                kdup[base:base + 64, kb * 128:(kb + 1) * 128],
                                qpair[h][base:base + 64,
                                         qt * 512 + off: qt * 512 + off + w],
                                start=True, stop=True,
                            )
                            if r >= 0:
                                nc.vector.tensor_add(sps[:, 0:128], sps[:, 0:128],
                                                     tri_sb)
                            ex = sbA.tile([128, 512], f32r, name=f"e{b}{h}{qt}{kb}",
                                          tag="exp", bufs=3)
                            nc.scalar.activation(ex[:, 0:w], sps[:, 0:w], Exp,
                                                 scale=0.125)
                            nc.tensor.matmul(
                                cps[:, off:off + w],
                                va[:, kb * 65:(kb + 1) * 65],
                                ex[:, 0:w],
                                start=(kb == 0), stop=(kb == nkb - 1),
                            )
                        # softmax normalization: recip of sums (row 64), bcast, mult
                        rc = sbA.tile([65, 512], f32r, name=f"rc{b}{h}{qt}", tag="rc",
                                      bufs=1)
                        with nc.allow_low_precision(reason="f32r softmax recip"):
                            nc.vector.reciprocal(rc[64:65, :], cps[64:65, :])
                        rbp = rb_psp.tile([64, 512], f32, name=f"rb{b}{h}{qt}",
                                          tag="rbvt")
                        nc.tensor.matmul(rbp[:], ones65[64:65, :], rc[64:65, :],
                                         start=True, stop=True)
                        rbs = sbA.tile([64, 512], f32, name=f"rbs{b}{h}{qt}", tag="rbs",
                                       bufs=1)
                        nc.vector.tensor_copy(rbs[:], rbp[:])
                        nc.vector.tensor_mul(ch[:, qt * 512:(qt + 1) * 512],
                                             cps[0:64, :], rbs[:])
                    if h == 1:
                        stage(b, 0)
                    elif h == 3:
                        stage(b, 1)

            def stage_a2a(b, p):
                # stage + exchange heads 2p, 2p+1 (f rows p*128:(p+1)*128 of each shard)
                for j in range(N_CORES):
                    for lane in range(2):
                        h = 2 * p + lane
                        nc.sync.dma_start(
                            cc_in[b][p][j, lane * 64:(lane + 1) * 64, :],
                            ctx_heads[(b, h)][:, j * TOK:(j + 1) * TOK],
                        )
                nc.gpsimd.collective_compute(
                    "AllToAll",
                    mybir.AluOpType.bypass,
                    replica_groups=[list(range(N_CORES))],
                    ins=[cc_in[b][p][:]],
                    outs=[cc_out[b][p][:]],
                )

            pre = tc.alloc_tile_pool(name="pre", bufs=1)
            woe0 = pre.tile([128, KC * 512], f32r, name="woe0")
            for fc in range(KC):
                nc.sync.dma_start(woe0[:, fc * 512:(fc + 1) * 512],
                                  woT[fc, :, 0:512])
            ccr0 = pre.tile([128, KC * TOK], f32r, name="ccr0")

            with tc.tile_pool(name="sbA", bufs=1) as sbA, \
                 tc.tile_pool(name="wp", bufs=1) as wpool, \
                 tc.tile_pool(name="xp", bufs=1) as xpool, \
                 tc.tile_pool(name="sc_ps", bufs=4, space="PSUM") as sc_ps, \
                 tc.tile_pool(name="cx_ps", bufs=2, space="PSUM") as cx_ps, \
                 tc.tile_pool(name="rb_ps", bufs=1, space="PSUM") as rb_psp:
                w_sb = wpool.tile([128, KC * 384], f32r, name="w_sb")
                for kc in range(KC):
                    nc.sync.dma_start(w_sb[:, kc * 384:(kc + 1) * 384], wqkvT[kc])
                q01, q23, kv0 = phase1(0, sbA, xpool)
                kdup0, va0 = phase15(0, kv0, sbA, rb_psp)
                attention(0, q01, q23, kdup0, va0, sbA, sc_ps, cx_ps, rb_psp,
                          stage_a2a)
                for fc in range(KC):
                    nc.sync.dma_start(ccr0[:, fc * TOK:(fc + 1) * TOK],
                                      cc_out[0][fc % 2][fc // 2, :, :])
                q01b, q23b, kv1 = phase1(1, sbA, xpool)
                kdup1, va1 = phase15(1, kv1, sbA, rb_psp)
                attention(1, q01b, q23b, kdup1, va1, sbA, sc_ps, cx_ps, rb_psp,
                          stage_a2a)

            # ---- out projection for local token slices ----
            with tc.tile_pool(name="sbB", bufs=1) as sbB, \
                 tc.tile_pool(name="op_ps", bufs=4, space="PSUM") as op_ps:
                woe = [woe0]
                for et in range(1, 4):
                    we = sbB.tile([128, KC * 512], f32r, name=f"woe{et}", tag="woe",
                                  bufs=3)
                    for fc in range(KC):
                        nc.sync.dma_start(we[:, fc * 512:(fc + 1) * 512],
                                          woT[fc, :, et * 512:(et + 1) * 512])
                    woe.append(we)
                t1 = sbB.tile([128, KC * TOK], f32r, name="cc1", tag="ccr", bufs=1)
                for fc in range(KC):
                    nc.sync.dma_start(t1[:, fc * TOK:(fc + 1) * TOK],
                                      cc_out[1][fc % 2][fc // 2, :, :])
                ctx_sb = {0: ccr0, 1: t1}
                for bb in range(B):
                    for et in range(4):
                        for mt in range(TOK // 128):
                            ps = op_ps.tile([128, 512], f32, name=f"o{bb}{et}{mt}",
                                            tag="op")
                            for fc in range(KC):
                                nc.tensor.matmul(
                                    ps[:],
                                    ctx_sb[bb][:, fc * TOK + mt * 128: fc * TOK + (mt + 1) * 128],
                                    woe[et][:, fc * 512:(fc + 1) * 512],
                                    start=(fc == 0), stop=(fc == KC - 1),
                                )
                            os_ = sbB.tile([128, 512], f32, name=f"os{bb}{et}{mt}",
                                           tag="os", bufs=2)
                            nc.vector.tensor_copy(os_[:], ps[:])
                            nc.sync.dma_start(
                                out[bb * TOK + mt * 128: bb * TOK + (mt + 1) * 128,
                                    et * 512:(et + 1) * 512],
                                os_[:],
                            )
            pre.release()

    nc.compile()
    return nc


def _prep_inputs(x, Wq, Wk, Wv, Wo):
    """Host-side sharding/transposes. Returns per-core in_maps."""
    xT = np.ascontiguousarray(
        x.transpose(0, 2, 1)).reshape(B, KC, 128, S).astype(np.float32)
    woT = np.ascontiguousarray(Wo.T).reshape(KC, 128, E).astype(np.float32)
    # additive causal mask for transposed scores: 0 where q>=kv (j>=i), -1e30 above
    tri = np.where(np.triu(np.ones((128, 128), dtype=np.float32)) > 0, 0.0,
                   -1e30).astype(np.float32)
    ident = np.eye(128, dtype=np.float32)
    ones = np.ones((128, 64), dtype=np.float32)
    misc = np.ascontiguousarray(np.concatenate([tri, ident, ones], axis=1))
    in_maps = []
    for c in range(N_CORES):
        wc = np.concatenate([
            Wq[256 * c:256 * (c + 1)],          # q heads 4c..4c+3 -> cols 0..255
            Wv[64 * c:64 * (c + 1)],            # v                -> cols 256..319
            Wk[64 * c:64 * (c + 1)],            # k                -> cols 320..383
        ], axis=0)                              # [384, E]
        wqkvT = np.ascontiguousarray(wc.T).reshape(KC, 128, 384).astype(np.float32)
        in_maps.append({"xT": xT, "wqkvT": wqkvT, "woT": woT, "misc": misc})
    return in_maps


def _ensure_ntff_hook():
    """Install antenv.axon_hooks shim so trace=True can capture NTFF profiles."""
    import sys
    import types
    try:
        from antenv.axon_hooks import get_axon_ntff_profile_hook  # noqa: F401
        return
    except ImportError:
        pass
    mod = types.ModuleType("antenv.axon_hooks")
    _h = [None]
    mod.set_axon_ntff_profile_hook = lambda h: _h.__setitem__(0, h)
    mod.get_axon_ntff_profile_hook = lambda: _h[0]
    sys.modules["antenv.axon_hooks"] = mod
    try:
        from trn_agent_boot.trn_boot import _ntff_profile_via_ctypes
        hook = _ntff_profile_via_ctypes("/opt/axon/libaxon_pjrt.so")
        if hook is not None:
            mod.set_axon_ntff_profile_hook(hook)
    except Exception:
        pass


def kernel(x, mask, Wq, Wk, Wv, Wo, trace=False):
    from concourse.bass_utils import run_bass_kernel_spmd
    if trace:
        _ensure_ntff_hook()

    x = np.asarray(x, dtype=np.float32)
    Wq = np.asarray(Wq, dtype=np.float32)
    Wk = np.asarray(Wk, dtype=np.float32)
    Wv = np.asarray(Wv, dtype=np.float32)
    Wo = np.asarray(Wo, dtype=np.float32)

    if "nc" not in _CACHE:
        _CACHE["nc"] = _build_nc()
    nc = _CACHE["nc"]

    in_maps = _prep_inputs(x, Wq, Wk, Wv, Wo)
    res = run_bass_kernel_spmd(nc, in_maps, core_ids=list(range(N_CORES)),
                               trace=trace)
    _CACHE["last_result"] = res

    full = np.empty((B, S, E), dtype=np.float32)
    for c in range(N_CORES):
        o = res.results[c]["out"]  # [B*TOK, E]
        for b in range(B):
            full[b, TOK * c:TOK * (c + 1), :] = o[b * TOK:(b + 1) * TOK]
    return full


# revision 26
# speedup vs baseline: 1.1849x; 1.0445x over previous
"""GQA multi-head attention (B=2, S=2048, E=2048, 32 q-heads, 8 kv-heads) on 8 TRN2 cores.

Sharding: tensor-parallel over kv-heads (core c owns kv-head c and query heads
4c..4c+3 for both batches). After attention, a per-batch AllToAll re-shards the
context from head-sharding to token-sharding; each core then runs the output
projection for its 256-token slice of each batch against the full Wo. The host
gather is pure concatenation.

All heavy matmuls run as float32r (full-rate PE, ~1e-4 rel err). Causality is
structural: above-diagonal score tiles are skipped, diagonal 128x128 blocks are
masked with a single precomputed triangular 0/1 mask. Softmax runs without
max-subtraction (causal scores are within [-12, 11] for this problem family);
row sums come from a ones-column appended to V inside the PV matmul, and the
1/sum normalization is broadcast across partitions with a K=1 matmul.
"""

import numpy as np

B = 2
S = 2048
E = 2048
KV = 8
G = 4
D = 64
H = 32
N_CORES = 8
KC = E // 128  # 16 contraction chunks
NT = S // 512  # 4 q-tiles of 512
NQ = 4         # token quarters for projections (512 each)
TOK = 256      # per-core token slice per batch (out proj)
NTB = S // 128  # 16 token blocks of 128

_CACHE = {}


def _build_nc():
    import concourse.mybir as mybir
    import concourse.tile as tile
    from concourse import bacc
    f32 = mybir.dt.float32
    f32r = mybir.dt.float32r
    bf16 = mybir.dt.bfloat16
    Exp = mybir.ActivationFunctionType.Exp

    nc = bacc.Bacc(target_bir_lowering=False, num_devices=N_CORES)

    xT = nc.dram_tensor("xT", [B, KC, 128, S], f32r, kind="ExternalInput")
    wqkvT = nc.dram_tensor("wqkvT", [KC, 128, 384], f32r, kind="ExternalInput")
    woT = nc.dram_tensor("woT", [KC, 128, E], f32r, kind="ExternalInput")
    misc = nc.dram_tensor("misc", [128, 320], f32r, kind="ExternalInput")
    out = nc.dram_tensor("out", [B * TOK, E], f32, kind="ExternalOutput")

    with tile.TileContext(nc) as tc:
        with tc.tile_pool(name="const", bufs=1) as const, \
             tc.tile_pool(name="mm_ps", bufs=1, space="PSUM") as mm_ps, \
             tc.tile_pool(name="dram", bufs=1, space="DRAM") as dram:

            # ---- constants ----
            misc_sb = const.tile([128, 320], f32r, name="misc_sb")
            nc.sync.dma_start(misc_sb[:], misc[:])
            tri_sb = misc_sb[:, 0:128]
            ident = misc_sb[:, 128:256]
            ones_col = misc_sb[:, 256:257]
            ones65 = misc_sb[:, 256:320]  # any all-ones block

            # ---- collective buffers (per batch) ----
            cc_in = [[dram.tile([N_CORES, 128, TOK], f32r, name=f"cc_in{b}{p}")
                      for p in range(2)] for b in range(B)]
            cc_out = [[dram.tile([N_CORES, 128, TOK], f32r, name=f"cc_out{b}{p}")
                       for p in range(2)] for b in range(B)]

            ctx_heads = {}  # (b, h) -> [64, S] sbuf tile

            def act_recip(out_ap, in_ap):
                # Reciprocal on ScalarE (LUT, ~2^-12 rel) — 5x cheaper than the
                # DVE Newton reciprocal; fine within this kernel's error budget.
                eng = nc.scalar
                ins_ = [eng.lower_ap(in_ap)] + [
                    mybir.ImmediateValue(dtype=mybir.dt.float32, value=v)
                    for v in (0.0, 1.0, 0.0)]
                return eng.add_instruction(mybir.InstActivation(
                    name=nc.get_next_instruction_name(),
                    func=mybir.ActivationFunctionType.Reciprocal,
                    ins=ins_, outs=[eng.lower_ap(out_ap)]))

            def phase1(b, sbA, xpool):
                """Projections for batch b -> q01/q23/kv sbuf tiles [128, S]."""
                q01 = sbA.tile([128, S], f32r, name=f"q01_{b}", tag="qkvT", bufs=3)
                q23 = sbA.tile([128, S], f32r, name=f"q23_{b}", tag="qkvT", bufs=3)
                kv = sbA.tile([128, S], f32r, name=f"kv_{b}", tag="qkvT", bufs=3)
                dst = [q01, q23, kv]
                for tq in range(NQ):  # token quarters of 512
                    xq = xpool.tile([128, KC * 512], f32r, name=f"x{b}{tq}",
                                    tag="x", bufs=2)
                    for kc in range(KC):
                        nc.sync.dma_start(
                            xq[:, kc * 512:(kc + 1) * 512],
                            xT[b, kc, :, tq * 512:(tq + 1) * 512])
                    for mc in range(3):
                        ps = mm_ps.tile([128, 512], f32, name=f"p{b}{tq}{mc}", tag="mm")
                        for kc in range(KC):
                            nc.tensor.matmul(
                                ps[:],
                                w_sb[:, kc * 384 + mc * 128: kc * 384 + (mc + 1) * 128],
                                xq[:, kc * 512:(kc + 1) * 512],
                                start=(kc == 0), stop=(kc == KC - 1),
                            )
                        nc.vector.tensor_copy(
                            dst[mc][:, tq * 512:(tq + 1) * 512], ps[:])
                return q01, q23, kv

            def phase15(b, kv, sbA, rb_psp):
                """kdup (k on both partition halves) + token-major v_aug tiles."""
                kdup = sbA.tile([128, S], f32r, name=f"kdup{b}", tag="kdup", bufs=1)
                # kv rows 64:128 hold k (layout [v; k])
                nc.sync.dma_start(kdup[0:64, :], kv[64:128, :])
                nc.sync.dma_start(kdup[64:128, :], kv[64:128, :])
                va = sbA.tile([128, NTB * 65], f32r, name=f"va{b}", tag="vaug", bufs=1)
                for t in range(NTB):
                    tp = rb_psp.tile([128, 64], f32r, name=f"vt{b}{t}", tag="rbvt")
                    nc.tensor.transpose(tp[:], kv[0:64, t * 128:(t + 1) * 128],
                                        misc_sb[0:64, 128:192])
                    nc.vector.tensor_copy(va[:, t * 65:t * 65 + 64], tp[:])
                    nc.vector.tensor_copy(va[:, t * 65 + 64:t * 65 + 65], ones_col)
                return kdup, va

            def attention(b, q01, q23, kdup, va, sbA, sc_ps, cx_ps, rb_psp, stage):
                qpair = [q01, q01, q23, q23]
                for h in range(G):
                    base = 64 * (h % 2)
                    ch = sbA.tile([64, S], f32r, name=f"ctx{b}{h}", tag="ctxh", bufs=3)
                    ctx_heads[(b, h)] = ch
                    for qt in range(NT):
                        cps = cx_ps.tile([65, 512], f32, name=f"c{b}{h}{qt}", tag="cx")
                        nkb = 4 * qt + 4
                        for kb in range(nkb):
                            r = kb - 4 * qt
                            off = 128 * r if r >= 0 else 0
                            w = 512 - off
                            sps = sc_ps.tile([128, 512], f32, name=f"s{b}{h}{qt}{kb}",
                                             tag="sc")
                            nc.tensor.matmul(
                                sps[:, 0:w],
                                kdup[base:base + 64, kb * 128:(kb + 1) * 128],
                                qpair[h][base:base + 64,
                                         qt * 512 + off: qt * 512 + off + w],
                                start=True, stop=True,
                            )
                            if r >= 0:
                                nc.vector.tensor_add(sps[:, 0:128], sps[:, 0:128],
                                                     tri_sb)
                            ex = sbA.tile([128, 512], f32r, name=f"e{b}{h}{qt}{kb}",
                                          tag="exp", bufs=3)
                            nc.scalar.activation(ex[:, 0:w], sps[:, 0:w], Exp,
                                                 scale=0.125)
                            nc.tensor.matmul(
                                cps[:, off:off + w],
                                va[:, kb * 65:(kb + 1) * 65],
                                ex[:, 0:w],
                                start=(kb == 0), stop=(kb == nkb - 1),
                            )
                        # softmax normalization: recip of sums (row 64), bcast, mult
                        rc = sbA.tile([65, 512], f32r, name=f"rc{b}{h}{qt}", tag="rc",
                                      bufs=1)
                        act_recip(rc[64:65, :], cps[64:65, :])
                        rbp = rb_psp.tile([64, 512], f32, name=f"rb{b}{h}{qt}",
                                          tag="rbvt")
                        nc.tensor.matmul(rbp[:], ones65[64:65, :], rc[64:65, :],
                                         start=True, stop=True)
                        rbs = sbA.tile([64, 512], f32, name=f"rbs{b}{h}{qt}", tag="rbs",
                                       bufs=1)
                        nc.vector.tensor_copy(rbs[:], rbp[:])
                        nc.vector.tensor_mul(ch[:, qt * 512:(qt + 1) * 512],
                                             cps[0:64, :], rbs[:])
                    if h == 1:
                        stage(b, 0)
                    elif h == 3:
                        stage(b, 1)

            def stage_a2a(b, p):
                # stage + exchange heads 2p, 2p+1 (f rows p*128:(p+1)*128 of each shard)
                for j in range(N_CORES):
                    for lane in range(2):
                        h = 2 * p + lane
                        nc.sync.dma_start(
                            cc_in[b][p][j, lane * 64:(lane + 1) * 64, :],
                            ctx_heads[(b, h)][:, j * TOK:(j + 1) * TOK],
                        )
                nc.gpsimd.collective_compute(
                    "AllToAll",
                    mybir.AluOpType.bypass,
                    replica_groups=[list(range(N_CORES))],
                    ins=[cc_in[b][p][:]],
                    outs=[cc_out[b][p][:]],
                )

            pre = tc.alloc_tile_pool(name="pre", bufs=1)
            woe0 = pre.tile([128, KC * 512], f32r, name="woe0")
            for fc in range(KC):
                nc.sync.dma_start(woe0[:, fc * 512:(fc + 1) * 512],
                                  woT[fc, :, 0:512])
            ccr0 = pre.tile([128, KC * TOK], f32r, name="ccr0")

            with tc.tile_pool(name="sbA", bufs=1) as sbA, \
                 tc.tile_pool(name="wp", bufs=1) as wpool, \
                 tc.tile_pool(name="xp", bufs=1) as xpool, \
                 tc.tile_pool(name="sc_ps", bufs=4, space="PSUM") as sc_ps, \
                 tc.tile_pool(name="cx_ps", bufs=2, space="PSUM") as cx_ps, \
                 tc.tile_pool(name="rb_ps", bufs=1, space="PSUM") as rb_psp:
                w_sb = wpool.tile([128, KC * 384], f32r, name="w_sb")
                for kc in range(KC):
                    nc.sync.dma_start(w_sb[:, kc * 384:(kc + 1) * 384], wqkvT[kc])
                q01, q23, kv0 = phase1(0, sbA, xpool)
                kdup0, va0 = phase15(0, kv0, sbA, rb_psp)
                attention(0, q01, q23, kdup0, va0, sbA, sc_ps, cx_ps, rb_psp,
                          stage_a2a)
                for fc in range(KC):
                    nc.sync.dma_start(ccr0[:, fc * TOK:(fc + 1) * TOK],
                                      cc_out[0][fc % 2][fc // 2, :, :])
                q01b, q23b, kv1 = phase1(1, sbA, xpool)
                kdup1, va1 = phase15(1, kv1, sbA, rb_psp)
                attention(1, q01b, q23b, kdup1, va1, sbA, sc_ps, cx_ps, rb_psp,
                          stage_a2a)

            # ---- out projection for local token slices ----
            with tc.tile_pool(name="sbB", bufs=1) as sbB, \
                 tc.tile_pool(name="op_ps", bufs=4, space="PSUM") as op_ps:
                woe = [woe0]
                for et in range(1, 4):
                    we = sbB.tile([128, KC * 512], f32r, name=f"woe{et}", tag="woe",
                                  bufs=3)
                    for fc in range(KC):
                        nc.sync.dma_start(we[:, fc * 512:(fc + 1) * 512],
                                          woT[fc, :, et * 512:(et + 1) * 512])
                    woe.append(we)
                t1 = sbB.tile([128, KC * TOK], f32r, name="cc1", tag="ccr", bufs=1)
                for fc in range(KC):
                    nc.sync.dma_start(t1[:, fc * TOK:(fc + 1) * TOK],
                                      cc_out[1][fc % 2][fc // 2, :, :])
                ctx_sb = {0: ccr0, 1: t1}
                for bb in range(B):
                    for et in range(4):
                        for mt in range(TOK // 128):
                            ps = op_ps.tile([128, 512], f32, name=f"o{bb}{et}{mt}",
                                            tag="op")
                            for fc in range(KC):
                                nc.tensor.matmul(
                                    ps[:],
                                    ctx_sb[bb][:, fc * TOK + mt * 128: fc * TOK + (mt + 1) * 128],
                                    woe[et][:, fc * 512:(fc + 1) * 512],
                                    start=(fc == 0), stop=(fc == KC - 1),
                                )
                            os_ = sbB.tile([128, 512], f32, name=f"os{bb}{et}{mt}",
                                           tag="os", bufs=2)
                            nc.vector.tensor_copy(os_[:], ps[:])
                            nc.sync.dma_start(
                                out[bb * TOK + mt * 128: bb * TOK + (mt + 1) * 128,
                                    et * 512:(et + 1) * 512],
                                os_[:],
                            )
            pre.release()

    nc.compile()
    return nc


def _prep_inputs(x, Wq, Wk, Wv, Wo):
    """Host-side sharding/transposes. Returns per-core in_maps."""
    xT = np.ascontiguousarray(
        x.transpose(0, 2, 1)).reshape(B, KC, 128, S).astype(np.float32)
    woT = np.ascontiguousarray(Wo.T).reshape(KC, 128, E).astype(np.float32)
    # additive causal mask for transposed scores: 0 where q>=kv (j>=i), -1e30 above
    tri = np.where(np.triu(np.ones((128, 128), dtype=np.float32)) > 0, 0.0,
                   -1e30).astype(np.float32)
    ident = np.eye(128, dtype=np.float32)
    ones = np.ones((128, 64), dtype=np.float32)
    misc = np.ascontiguousarray(np.concatenate([tri, ident, ones], axis=1))
    in_maps = []
    for c in range(N_CORES):
        wc = np.concatenate([
            Wq[256 * c:256 * (c + 1)],          # q heads 4c..4c+3 -> cols 0..255
            Wv[64 * c:64 * (c + 1)],            # v                -> cols 256..319
            Wk[64 * c:64 * (c + 1)],            # k                -> cols 320..383
        ], axis=0)                              # [384, E]
        wqkvT = np.ascontiguousarray(wc.T).reshape(KC, 128, 384).astype(np.float32)
        in_maps.append({"xT": xT, "wqkvT": wqkvT, "woT": woT, "misc": misc})
    return in_maps


def _ensure_ntff_hook():
    """Install antenv.axon_hooks shim so trace=True can capture NTFF profiles."""
    import sys
    import types
    try:
        from antenv.axon_hooks import get_axon_ntff_profile_hook  # noqa: F401
        return
    except ImportError:
        pass
    mod = types.ModuleType("antenv.axon_hooks")
    _h = [None]
    mod.set_axon_ntff_profile_hook = lambda h: _h.__setitem__(0, h)
    mod.get_axon_ntff_profile_hook = lambda: _h[0]
    sys.modules["antenv.axon_hooks"] = mod
    try:
        from trn_agent_boot.trn_boot import _ntff_profile_via_ctypes
        hook = _ntff_profile_via_ctypes("/opt/axon/libaxon_pjrt.so")
        if hook is not None:
            mod.set_axon_ntff_profile_hook(hook)
    except Exception:
        pass


def kernel(x, mask, Wq, Wk, Wv, Wo, trace=False):
    from concourse.bass_utils import run_bass_kernel_spmd
    if trace:
        _ensure_ntff_hook()

    x = np.asarray(x, dtype=np.float32)
    Wq = np.asarray(Wq, dtype=np.float32)
    Wk = np.asarray(Wk, dtype=np.float32)
    Wv = np.asarray(Wv, dtype=np.float32)
    Wo = np.asarray(Wo, dtype=np.float32)

    if "nc" not in _CACHE:
        _CACHE["nc"] = _build_nc()
    nc = _CACHE["nc"]

    in_maps = _prep_inputs(x, Wq, Wk, Wv, Wo)
    res = run_bass_kernel_spmd(nc, in_maps, core_ids=list(range(N_CORES)),
                               trace=trace)
    _CACHE["last_result"] = res

    full = np.empty((B, S, E), dtype=np.float32)
    for c in range(N_CORES):
        o = res.results[c]["out"]  # [B*TOK, E]
        for b in range(B):
            full[b, TOK * c:TOK * (c + 1), :] = o[b * TOK:(b + 1) * TOK]
    return full
